# revision 23
# baseline (speedup 1.0000x reference)
"""DSSM (S4D-style) FFT-convolution kernel for Trainium2, 8 NeuronCores.

Math: y[b,h,:] = causal_conv(u_masked[b,h,:], K[h,:]) + D[h]*u_masked, masked,
where K[h,l] = 2*Re(sum_n Cs[h,n] * w[h,n]^l), w = exp(dt*A).

Algorithm (chunked state-space, T=256, J=16 chunks, N=64 complex states):
  intra-chunk:  lower-triangular Toeplitz matmul with K[0:256] (D folded in)
  inter-chunk:  A_j = V^T u_j (Vandermonde projection), S_j = w^T.S_{j-1} + A_j
                (complex scan over 16 steps on the vector engine, bf16),
                y_inter = Wout^T S_{j-1}
Sharding: H=256 channels split across 8 cores (32 each). Host does masking,
batch length-sorting, layout transforms, bf16 casts, and final unshard+mask.
Ragged lengths: batch sorted by length desc; scan steps and output stores
slice/skip the dead suffix.
"""

import numpy as np
import ml_dtypes

import concourse.bass as bass
import concourse.bacc as bacc
import concourse.mybir as mybir
import concourse.tile as tile
from concourse.bass_utils import run_bass_kernel_spmd

H, N, B, L = 256, 64, 16, 4096
NCORES = 8
HC = H // NCORES            # 32 channels per core
T, J = 256, 16              # chunk length, number of chunks
JH = 2                      # j-halves (8 chunks each) for 128-row tiles
N2 = 2 * N                  # 128 (real+imag state rows)
HG = 8                      # h-groups for pipelined input DMA

F32 = mybir.dt.float32
BF16 = mybir.dt.bfloat16
NP_BF16 = ml_dtypes.bfloat16


def _build_program(k_b):
    """k_b: per-(sorted)batch chunk counts, used to skip dead work at trace
    time. Correctness does not depend on them (host masks dead regions)."""
    # number of batches needing scan step j (producing S_j, used by chunk j+1)
    nb_scan = [sum(1 for k in k_b if k > j + 1) for j in range(J)]
    # number of batches with any alive chunk in half jh
    nb_half = [sum(1 for k in k_b if k > jh * 8) for jh in range(JH)]

    nc = bacc.Bacc("TRN2", target_bir_lowering=False, debug=False,
                   enable_asserts=False, num_devices=NCORES)

    HCG = HC // HG
    u_d = nc.dram_tensor("u_arr", [128, HC * 2 * J * B], BF16,
                         kind="ExternalInput")
    v_d = nc.dram_tensor("vwts", [128, HC * 256], BF16, kind="ExternalInput")
    w_d = nc.dram_tensor("wts", [128, HC * 512], BF16, kind="ExternalInput")
    pre_d = nc.dram_tensor("p_re", [128, HC * B], BF16, kind="ExternalInput")
    pim_d = nc.dram_tensor("p_im_s", [128, HC * B], BF16,
                           kind="ExternalInput")
    y_d = nc.dram_tensor("y", [HC, 128, JH * T], BF16, kind="ExternalOutput")

    with tile.TileContext(nc) as tc:
        with (
            tc.tile_pool(name="const", bufs=1) as cpool,
            tc.tile_pool(name="scantmp", bufs=2) as spool,
            tc.tile_pool(name="ysb", bufs=8) as ypool,
            tc.tile_pool(name="psum", bufs=8, space="PSUM") as psum,
        ):
            u_t = cpool.tile([128, HC * 2 * J * B], BF16, name="u_t")
            v_t = cpool.tile([128, HC * 256], BF16, name="v_t")
            w_t = cpool.tile([128, HC * 512], BF16, name="w_t")
            pre_t = cpool.tile([128, HC * B], BF16, name="pre_t")
            pim_t = cpool.tile([128, HC * B], BF16, name="pim_t")
            a_all = cpool.tile([128, HC * J * B], BF16, name="a_all")
            s_all = cpool.tile([128, HC * J * B], BF16, name="s_all")

            # split input DMAs by h-group so phase A can start early;
            # phase-C weights (Qrow/Wout) arrive last
            ucols = HCG * 2 * J * B
            vcols = HCG * 256
            for g in range(HG):
                nc.sync.dma_start(v_t[:, g * vcols:(g + 1) * vcols],
                                  v_d[:, g * vcols:(g + 1) * vcols])
                nc.sync.dma_start(u_t[:, g * ucols:(g + 1) * ucols],
                                  u_d[:, g * ucols:(g + 1) * ucols])
            nc.sync.dma_start(pre_t[:], pre_d[:])
            nc.sync.dma_start(pim_t[:], pim_d[:])
            wcols = HCG * 512
            for g in range(HG):
                nc.sync.dma_start(w_t[:, g * wcols:(g + 1) * wcols],
                                  w_d[:, g * wcols:(g + 1) * wcols])

            # views
            # A store: same (h, jh, b, jl) layout as s_all
            a_v = a_all[:].rearrange("p (h jh b jl) -> p h jh b jl",
                                     h=HC, jh=JH, b=B)
            # state store: col = ((h*2 + jh)*B + b)*8 + jl, slot = jh*8+jl
            s_v = s_all[:].rearrange("p (h jh b jl) -> p h jh b jl",
                                     h=HC, jh=JH, b=B)

            def vslice(h, lo, hi):
                return v_t[:, h * 256 + lo:h * 256 + hi]

            def wslice(h, lo, hi):
                return w_t[:, h * 512 + lo:h * 512 + hi]

            # zero states (slot 0 = S_{-1}; also keeps skipped slots clean)
            nc.gpsimd.memset(s_all[:], 0.0)

            # ---- Phase A: state input projections A_j = V^T u_j ----
            for hp in range(HC // 2):
                a_ps = psum.tile([128, 2 * J * B], F32, name="a_ps",
                                 tag="ps")
                for i in range(2):
                    h = 2 * hp + i
                    nc.tensor.matmul(
                        a_ps[:, i * J * B:(i + 1) * J * B],
                        vslice(h, 0, 128),
                        u_t[:, (h * 2 + 0) * J * B:(h * 2 + 1) * J * B],
                        start=True, stop=False, skip_group_check=(i == 1))
                    nc.tensor.matmul(
                        a_ps[:, i * J * B:(i + 1) * J * B],
                        vslice(h, 128, 256),
                        u_t[:, (h * 2 + 1) * J * B:(h * 2 + 2) * J * B],
                        start=False, stop=True, skip_group_check=(i == 1))
                # psum col order == a_all col order: contiguous cast copy
                nc.scalar.copy(a_all[:, hp * 512:(hp + 1) * 512], a_ps[:])

            # ---- Phase B: complex scan S_j = p * S_{j-1} + A_j (bf16) ----
            pre_v = pre_t[:].rearrange("p (h b) -> p h b", h=HC)
            pim_v = pim_t[:].rearrange("p (h b) -> p h b", h=HC)
            for j in range(J):
                nb = nb_scan[j]
                if nb == 0:
                    continue
                jh_r, jl_r = divmod(j, 8)
                jh_r2, jl_r2 = jh_r, jl_r
                jh_w, jl_w = divmod(j + 1, 8)
                s_in = s_v[:, :, jh_r, 0:nb, jl_r]
                m_a = spool.tile([128, HC, B], BF16, name="m_a")
                swp = spool.tile([128, HC, B], BF16, name="swp")
                m_b = spool.tile([128, HC, B], BF16, name="m_b")
                tt = spool.tile([128, HC, B], BF16, name="tt")
                nc.vector.tensor_mul(m_a[:, :, 0:nb], pre_v[:, :, 0:nb], s_in)
                nc.vector.tensor_copy(swp[0:64, :, 0:nb], s_in[64:128])
                nc.vector.tensor_copy(swp[64:128, :, 0:nb], s_in[0:64])
                nc.vector.tensor_mul(m_b[:, :, 0:nb], pim_v[:, :, 0:nb],
                                     swp[:, :, 0:nb])
                nc.vector.tensor_add(tt[:, :, 0:nb], m_a[:, :, 0:nb],
                                     m_b[:, :, 0:nb])
                nc.vector.tensor_add(s_v[:, :, jh_w, 0:nb, jl_w],
                                     tt[:, :, 0:nb],
                                     a_v[:, :, jh_r2, 0:nb, jl_r2])

            # ---- Phase C: intra Toeplitz + state output projection ----
            # stores merged 4 h at a time (dma_start issue cost dominates
            # many small stores)
            for jh in range(JH):
                rows = min(128, 8 * nb_half[jh])
                if rows == 0:
                    continue
                for hg in range(HC // 4):
                    y_sb = ypool.tile([128, 4, T], BF16, name="y_sb")
                    for i in range(4):
                        h = hg * 4 + i
                        y_ps = psum.tile([128, T], F32, name="y_ps",
                                         tag="ps")
                        # lhsT cols ordered (b, jl) -> out partitions b*8+jl
                        uo0 = ((h * 2 + 0) * 2 + jh) * 128
                        uo1 = ((h * 2 + 1) * 2 + jh) * 128
                        so = (h * 2 + jh) * 128
                        nc.tensor.matmul(y_ps[:], u_t[:, uo0:uo0 + 128],
                                         wslice(h, 0, 256),
                                         start=True, stop=False)
                        nc.tensor.matmul(y_ps[:, 128:256],
                                         u_t[:, uo1:uo1 + 128],
                                         wslice(h, 0, 128),
                                         start=False, stop=False,
                                         skip_group_check=True)
                        nc.tensor.matmul(y_ps[:], s_all[:, so:so + 128],
                                         wslice(h, 256, 512),
                                         start=False, stop=True,
                                         skip_group_check=True)
                        if (h + jh) % 2 == 0:
                            nc.scalar.copy(y_sb[0:rows, i, :], y_ps[0:rows])
                        else:
                            nc.vector.tensor_copy(y_sb[0:rows, i, :],
                                                  y_ps[0:rows])
                    # dst iteration (row, h, tau) to match src layout
                    dst = y_d[hg * 4:hg * 4 + 4, 0:rows,
                              jh * T:(jh + 1) * T].rearrange(
                                  "h r t -> r h t")
                    nc.sync.dma_start(dst, y_sb[0:rows])

    nc.compile()
    return nc


_CACHE = {}


def _get_program(k_b):
    key = tuple(k_b)
    if key not in _CACHE:
        _CACHE[key] = _build_program(k_b)
    return _CACHE[key]


def _host_precompute(log_dt, C, log_A_real, A_imag, D):
    """Returns per-h weight blocks (fp64 internally)."""
    dt = np.exp(log_dt.astype(np.float64))
    A = -np.exp(log_A_real.astype(np.float64)) + 1j * A_imag.astype(np.float64)
    dtA = A * dt[:, None]
    w = np.exp(dtA)                                   # (H,N)
    Cc = C[..., 0].astype(np.float64) + 1j * C[..., 1].astype(np.float64)
    Cs = Cc * (np.exp(dtA) - 1.0) / A                 # (H,N)

    l = np.arange(T, dtype=np.float64)
    K = 2.0 * np.einsum('hn,hnl->hl', Cs, np.exp(dtA[:, :, None] * l)).real
    K[:, 0] += D.astype(np.float64)

    sig = np.arange(T)
    Vc = w[:, None, :] ** (T - sig)[None, :, None]    # (H,T,N)
    V_real = np.concatenate([Vc.real, Vc.imag], axis=2)  # (H,T,2N)

    tau = np.arange(T)
    Wc = Cs[:, :, None] * w[:, :, None] ** tau        # (H,N,T)
    W_real = np.concatenate([2 * Wc.real, -2 * Wc.imag], axis=1)  # (H,2N,T)

    p = w ** T                                        # (H,N)

    # Qrow0[h, sig', tau] = K[h, tau - sig'] for tau >= sig' else 0
    sp = np.arange(128)
    d = tau[None, :] - sp[:, None]                    # (128, 256)
    Qrow0 = np.where((d >= 0)[None], K[:, np.clip(d, 0, T - 1)], 0.0)
    return Qrow0, V_real, W_real, p


def kernel(u, length, log_dt, C, log_A_real, A_imag, D, **_unused):
    u = np.asarray(u, dtype=np.float32)
    length = np.asarray(length).astype(np.int64)
    mask = (np.arange(L)[None, :] < length[:, None])
    u_m = (u * mask[:, None, :]).astype(np.float32)

    # sort batches by length desc (stable) so dead work is a suffix
    perm = np.argsort(-length, kind="stable")
    k_b = [int(min(J, (int(length[b]) + T - 1) // T)) for b in perm]

    Qrow0, V_real, W_real, p = _host_precompute(
        np.asarray(log_dt), np.asarray(C), np.asarray(log_A_real),
        np.asarray(A_imag), np.asarray(D))

    # phase-A weights per h: [128, 256] = [V0 | V1]; phase-C: [Qrow0 | Wout]
    vwts = np.empty((H, 128, 256), dtype=np.float64)
    vwts[:, :, 0:128] = V_real[:, 0:128, :]      # lhsT [sig', n2]
    vwts[:, :, 128:256] = V_real[:, 128:256, :]
    vwts = vwts.astype(NP_BF16)
    wts = np.empty((H, 128, 512), dtype=np.float64)
    wts[:, :, 0:256] = Qrow0
    wts[:, :, 256:512] = W_real                  # rhs [n2, tau]
    wts = wts.astype(NP_BF16)

    # p tiles: [n2, (h,b)] with re duplicated on both halves; im sign-split
    p_re = np.empty((128, H), dtype=np.float32)
    p_im = np.empty((128, H), dtype=np.float32)
    p_re[0:64] = p.real.T
    p_re[64:128] = p.real.T
    p_im[0:64] = -p.imag.T
    p_im[64:128] = p.imag.T

    # u layout: (b,h,jh,jl,sb,sig') -> [sig', h, sb, jh, b_sorted, jl]
    u_s = u_m[perm]
    u_r = u_s.reshape(B, H, JH, 8, 2, 128).transpose(5, 1, 4, 2, 0, 3)
    u_bf = np.ascontiguousarray(u_r).astype(NP_BF16)

    nc = _get_program(k_b)
    in_maps = []
    for c in range(NCORES):
        hs = slice(c * HC, (c + 1) * HC)
        in_maps.append({
            "u_arr": np.ascontiguousarray(
                u_bf[:, hs].reshape(128, HC * 2 * J * B)),
            "vwts": np.ascontiguousarray(
                vwts[hs].transpose(1, 0, 2).reshape(128, HC * 256)),
            "wts": np.ascontiguousarray(
                wts[hs].transpose(1, 0, 2).reshape(128, HC * 512)),
            "p_re": np.ascontiguousarray(
                np.repeat(p_re[:, hs], B, axis=1)).astype(NP_BF16),
            "p_im_s": np.ascontiguousarray(
                np.repeat(p_im[:, hs], B, axis=1)).astype(NP_BF16),
        })

    res = run_bass_kernel_spmd(nc, in_maps, core_ids=list(range(NCORES)))

    y = np.empty((B, H, L), dtype=np.float32)
    for c in range(NCORES):
        yc = res.results[c]["y"].astype(np.float32)  # [HC, 128, 512]
        yc = yc.reshape(HC, B, 8, JH, T)             # (h, b, jl, jh, tau)
        yc = yc.transpose(1, 0, 3, 2, 4).reshape(B, HC, L)  # l=(jh,jl,tau)
        y[:, c * HC:(c + 1) * HC, :] = yc
    inv = np.empty(B, dtype=np.int64)
    inv[perm] = np.arange(B)
    y = y[inv]
    # np.where (not multiply): skipped store regions may hold garbage
    # bits (incl. NaN) when output buffers arrive non-zeroed
    y = np.where(mask[:, None, :], y, np.float32(0.0))
    return np.ascontiguousarray(y)


# revision 28
# speedup vs baseline: 1.0443x; 1.0443x over previous
"""DSSM (S4D-style) FFT-convolution kernel for Trainium2, 8 NeuronCores.

Math: y[b,h,:] = causal_conv(u_masked[b,h,:], K[h,:]) + D[h]*u_masked, masked,
where K[h,l] = 2*Re(sum_n Cs[h,n] * w[h,n]^l), w = exp(dt*A).

Algorithm (chunked state-space, T=256, J=16 chunks, N=64 complex states):
  intra-chunk:  lower-triangular Toeplitz matmul with K[0:256] (D folded in)
  inter-chunk:  A_j = V^T u_j (Vandermonde projection), S_j = w^T.S_{j-1} + A_j
                (complex scan over 16 steps on the vector engine, bf16),
                y_inter = Wout^T S_{j-1}
Sharding: H=256 channels split across 8 cores (32 each). Host does masking,
batch length-sorting, layout transforms, bf16 casts, and final unshard+mask.
Ragged lengths: batch sorted by length desc; scan steps and output stores
slice/skip the dead suffix.
"""

import numpy as np
import ml_dtypes

import concourse.bass as bass
import concourse.bacc as bacc
import concourse.mybir as mybir
import concourse.tile as tile
from concourse.bass_utils import run_bass_kernel_spmd

H, N, B, L = 256, 64, 16, 4096
NCORES = 8
HC = H // NCORES            # 32 channels per core
T, J = 256, 16              # chunk length, number of chunks
JH = 2                      # j-halves (8 chunks each) for 128-row tiles
N2 = 2 * N                  # 128 (real+imag state rows)
HG = 8                      # h-groups for pipelined input DMA

F32 = mybir.dt.float32
BF16 = mybir.dt.bfloat16
NP_BF16 = ml_dtypes.bfloat16


def _build_program(k_b):
    """k_b: per-(sorted)batch chunk counts, used to skip dead work at trace
    time. Correctness does not depend on them (host masks dead regions)."""
    # number of batches needing scan step j (producing S_j, used by chunk j+1)
    nb_scan = [sum(1 for k in k_b if k > j + 1) for j in range(J)]
    # number of batches with any alive chunk in half jh
    nb_half = [sum(1 for k in k_b if k > jh * 8) for jh in range(JH)]

    nc = bacc.Bacc("TRN2", target_bir_lowering=False, debug=False,
                   enable_asserts=False, num_devices=NCORES)

    HCG = HC // HG
    u_d = nc.dram_tensor("u_arr", [128, HC * 2 * J * B], BF16,
                         kind="ExternalInput")
    v_d = nc.dram_tensor("vwts", [128, HC * 256], BF16, kind="ExternalInput")
    w_d = nc.dram_tensor("wts", [128, HC * 512], BF16, kind="ExternalInput")
    pre_d = nc.dram_tensor("p_re", [128, HC * B], BF16, kind="ExternalInput")
    pim_d = nc.dram_tensor("p_im_s", [128, HC * B], BF16,
                           kind="ExternalInput")
    y_d = nc.dram_tensor("y", [HC, 128, JH * T], BF16, kind="ExternalOutput")

    with tile.TileContext(nc) as tc:
        with (
            tc.tile_pool(name="const", bufs=1) as cpool,
            tc.tile_pool(name="scantmp", bufs=2) as spool,
            tc.tile_pool(name="ysb", bufs=8) as ypool,
            tc.tile_pool(name="psum", bufs=8, space="PSUM") as psum,
        ):
            u_t = cpool.tile([128, HC * 2 * J * B], BF16, name="u_t")
            v_t = cpool.tile([128, HC * 256], BF16, name="v_t")
            w_t = cpool.tile([128, HC * 512], BF16, name="w_t")
            pre_t = cpool.tile([128, HC * B], BF16, name="pre_t")
            pim_t = cpool.tile([128, HC * B], BF16, name="pim_t")
            a_all = cpool.tile([128, HC * J * B], BF16, name="a_all")
            s_all = cpool.tile([128, (J + 1) * HC * B], BF16,
                               name="s_all")
            s_yout = cpool.tile([128, HC * J * B], BF16, name="s_yout")

            # split input DMAs by h-group so phase A can start early;
            # phase-C weights (Qrow/Wout) arrive last
            ucols = HCG * 2 * J * B
            vcols = HCG * 256
            for g in range(HG):
                nc.sync.dma_start(v_t[:, g * vcols:(g + 1) * vcols],
                                  v_d[:, g * vcols:(g + 1) * vcols])
                nc.sync.dma_start(u_t[:, g * ucols:(g + 1) * ucols],
                                  u_d[:, g * ucols:(g + 1) * ucols])
            nc.sync.dma_start(pre_t[:], pre_d[:])
            nc.sync.dma_start(pim_t[:], pim_d[:])
            wcols = HCG * 512
            for g in range(HG):
                nc.sync.dma_start(w_t[:, g * wcols:(g + 1) * wcols],
                                  w_d[:, g * wcols:(g + 1) * wcols])

            # views: a_all/s_all are SLOT-major (slot, h, b) so every scan
            # op is contiguous step-1 (2x/4x DVE modes); s_yout is
            # (h, jh, b, jl) for contiguous phase-C lhsT quarters
            SW = HC * B
            a_sl = a_all[:].rearrange("p (sl b h) -> p sl b h", sl=J, b=B)
            sy_v = s_yout[:].rearrange("p (h jh b jl) -> p h jh b jl",
                                       h=HC, jh=JH, b=B)

            def vslice(h, lo, hi):
                return v_t[:, h * 256 + lo:h * 256 + hi]

            def wslice(h, lo, hi):
                return w_t[:, h * 512 + lo:h * 512 + hi]

            # zero states (slot 0 = S_{-1}; also keeps skipped slots clean)
            nc.gpsimd.memset(s_all[:], 0.0)
            nc.gpsimd.memset(s_yout[:], 0.0)

            # ---- Phase A: state input projections A_j = V^T u_j ----
            for hp in range(HC // 2):
                a_ps = psum.tile([128, 2 * J * B], F32, name="a_ps",
                                 tag="ps")
                for i in range(2):
                    h = 2 * hp + i
                    nc.tensor.matmul(
                        a_ps[:, i * J * B:(i + 1) * J * B],
                        vslice(h, 0, 128),
                        u_t[:, (h * 2 + 0) * J * B:(h * 2 + 1) * J * B],
                        start=True, stop=False, skip_group_check=(i == 1))
                    nc.tensor.matmul(
                        a_ps[:, i * J * B:(i + 1) * J * B],
                        vslice(h, 128, 256),
                        u_t[:, (h * 2 + 1) * J * B:(h * 2 + 2) * J * B],
                        start=False, stop=True, skip_group_check=(i == 1))
                # scatter psum cols (i, jh, b, jl) into slot-major a_all;
                # split per (i, jh) to stay within 3D access patterns
                for i in range(2):
                    for jjh in range(JH):
                        a_out = a_sl[:, jjh * 8:(jjh + 1) * 8, :,
                                     2 * hp + i].rearrange("p jl b -> p b jl")
                        a_in = a_ps[:, i * 256 + jjh * 128:
                                    i * 256 + jjh * 128 + 128].rearrange(
                            "p (b jl) -> p b jl", b=B)
                        if (i + jjh) % 2 == 0:
                            nc.scalar.copy(a_out, a_in)
                        else:
                            nc.vector.tensor_copy(a_out, a_in)

            # ---- Phase B: complex scan S_j = p * S_{j-1} + A_j (bf16) ----
            for j in range(J):
                if nb_scan[j] == 0:
                    continue
                w_ = 32 * nb_scan[j]
                s_in = s_all[:, j * SW:j * SW + w_]
                s_out = s_all[:, (j + 1) * SW:(j + 1) * SW + w_]
                m_a = spool.tile([128, SW], BF16, name="m_a")
                swp = spool.tile([128, SW], BF16, name="swp")
                m_b = spool.tile([128, SW], BF16, name="m_b")
                tt = spool.tile([128, SW], BF16, name="tt")
                nc.vector.tensor_mul(m_a[:, 0:w_], pre_t[:, 0:w_], s_in)
                nc.vector.tensor_copy(swp[0:64, 0:w_], s_in[64:128])
                nc.vector.tensor_copy(swp[64:128, 0:w_], s_in[0:64])
                nc.vector.tensor_mul(m_b[:, 0:w_], pim_t[:, 0:w_], swp[:, 0:w_])
                nc.vector.tensor_add(tt[:, 0:w_], m_a[:, 0:w_], m_b[:, 0:w_])
                nc.vector.tensor_add(s_out, tt[:, 0:w_],
                                     a_all[:, j * SW:j * SW + w_])
                # off-chain: mirror the new slot into the yout layout
                jh_w, jl_w = divmod(j + 1, 8)
                if j + 1 < J:
                    nc.scalar.copy(
                        sy_v[:, :, jh_w, 0:nb_scan[j], jl_w],
                        s_out.rearrange("p (b h) -> p h b", b=nb_scan[j]))

            # ---- Phase C: intra Toeplitz + state output projection ----
            # stores merged 4 h at a time (dma_start issue cost dominates
            # many small stores)
            for jh in range(JH):
                rows = min(128, 8 * nb_half[jh])
                if rows == 0:
                    continue
                for hg in range(HC // 4):
                    y_sb = ypool.tile([128, 4, T], BF16, name="y_sb")
                    for i in range(4):
                        h = hg * 4 + i
                        y_ps = psum.tile([128, T], F32, name="y_ps",
                                         tag="ps")
                        # lhsT cols ordered (b, jl) -> out partitions b*8+jl
                        uo0 = ((h * 2 + 0) * 2 + jh) * 128
                        uo1 = ((h * 2 + 1) * 2 + jh) * 128
                        so = (h * 2 + jh) * 128
                        nc.tensor.matmul(y_ps[:], u_t[:, uo0:uo0 + 128],
                                         wslice(h, 0, 256),
                                         start=True, stop=False)
                        nc.tensor.matmul(y_ps[:, 128:256],
                                         u_t[:, uo1:uo1 + 128],
                                         wslice(h, 0, 128),
                                         start=False, stop=False,
                                         skip_group_check=True)
                        nc.tensor.matmul(y_ps[:], s_yout[:, so:so + 128],
                                         wslice(h, 256, 512),
                                         start=False, stop=True,
                                         skip_group_check=True)
                        if (h + jh) % 2 == 0:
                            nc.scalar.copy(y_sb[0:rows, i, :], y_ps[0:rows])
                        else:
                            nc.vector.tensor_copy(y_sb[0:rows, i, :],
                                                  y_ps[0:rows])
                    # dst iteration (row, h, tau) to match src layout
                    dst = y_d[hg * 4:hg * 4 + 4, 0:rows,
                              jh * T:(jh + 1) * T].rearrange(
                                  "h r t -> r h t")
                    nc.sync.dma_start(dst, y_sb[0:rows])

    nc.compile()
    return nc


_CACHE = {}


def _get_program(k_b):
    key = tuple(k_b)
    if key not in _CACHE:
        _CACHE[key] = _build_program(k_b)
    return _CACHE[key]


def _host_precompute(log_dt, C, log_A_real, A_imag, D):
    """Returns per-h weight blocks (fp64 internally)."""
    dt = np.exp(log_dt.astype(np.float64))
    A = -np.exp(log_A_real.astype(np.float64)) + 1j * A_imag.astype(np.float64)
    dtA = A * dt[:, None]
    w = np.exp(dtA)                                   # (H,N)
    Cc = C[..., 0].astype(np.float64) + 1j * C[..., 1].astype(np.float64)
    Cs = Cc * (np.exp(dtA) - 1.0) / A                 # (H,N)

    l = np.arange(T, dtype=np.float64)
    K = 2.0 * np.einsum('hn,hnl->hl', Cs, np.exp(dtA[:, :, None] * l)).real
    K[:, 0] += D.astype(np.float64)

    sig = np.arange(T)
    Vc = w[:, None, :] ** (T - sig)[None, :, None]    # (H,T,N)
    V_real = np.concatenate([Vc.real, Vc.imag], axis=2)  # (H,T,2N)

    tau = np.arange(T)
    Wc = Cs[:, :, None] * w[:, :, None] ** tau        # (H,N,T)
    W_real = np.concatenate([2 * Wc.real, -2 * Wc.imag], axis=1)  # (H,2N,T)

    p = w ** T                                        # (H,N)

    # Qrow0[h, sig', tau] = K[h, tau - sig'] for tau >= sig' else 0
    sp = np.arange(128)
    d = tau[None, :] - sp[:, None]                    # (128, 256)
    Qrow0 = np.where((d >= 0)[None], K[:, np.clip(d, 0, T - 1)], 0.0)
    return Qrow0, V_real, W_real, p


def kernel(u, length, log_dt, C, log_A_real, A_imag, D, **_unused):
    u = np.asarray(u, dtype=np.float32)
    length = np.asarray(length).astype(np.int64)
    mask = (np.arange(L)[None, :] < length[:, None])
    u_m = (u * mask[:, None, :]).astype(np.float32)

    # sort batches by length desc (stable) so dead work is a suffix
    perm = np.argsort(-length, kind="stable")
    k_b = [int(min(J, (int(length[b]) + T - 1) // T)) for b in perm]

    Qrow0, V_real, W_real, p = _host_precompute(
        np.asarray(log_dt), np.asarray(C), np.asarray(log_A_real),
        np.asarray(A_imag), np.asarray(D))

    # phase-A weights per h: [128, 256] = [V0 | V1]; phase-C: [Qrow0 | Wout]
    vwts = np.empty((H, 128, 256), dtype=np.float64)
    vwts[:, :, 0:128] = V_real[:, 0:128, :]      # lhsT [sig', n2]
    vwts[:, :, 128:256] = V_real[:, 128:256, :]
    vwts = vwts.astype(NP_BF16)
    wts = np.empty((H, 128, 512), dtype=np.float64)
    wts[:, :, 0:256] = Qrow0
    wts[:, :, 256:512] = W_real                  # rhs [n2, tau]
    wts = wts.astype(NP_BF16)

    # p tiles: [n2, (h,b)] with re duplicated on both halves; im sign-split
    p_re = np.empty((128, H), dtype=np.float32)
    p_im = np.empty((128, H), dtype=np.float32)
    p_re[0:64] = p.real.T
    p_re[64:128] = p.real.T
    p_im[0:64] = -p.imag.T
    p_im[64:128] = p.imag.T

    # u layout: (b,h,jh,jl,sb,sig') -> [sig', h, sb, jh, b_sorted, jl]
    u_s = u_m[perm]
    u_r = u_s.reshape(B, H, JH, 8, 2, 128).transpose(5, 1, 4, 2, 0, 3)
    u_bf = np.ascontiguousarray(u_r).astype(NP_BF16)

    nc = _get_program(k_b)
    in_maps = []
    for c in range(NCORES):
        hs = slice(c * HC, (c + 1) * HC)
        in_maps.append({
            "u_arr": np.ascontiguousarray(
                u_bf[:, hs].reshape(128, HC * 2 * J * B)),
            "vwts": np.ascontiguousarray(
                vwts[hs].transpose(1, 0, 2).reshape(128, HC * 256)),
            "wts": np.ascontiguousarray(
                wts[hs].transpose(1, 0, 2).reshape(128, HC * 512)),
            "p_re": np.ascontiguousarray(
                np.tile(p_re[:, hs], (1, B))).astype(NP_BF16),
            "p_im_s": np.ascontiguousarray(
                np.tile(p_im[:, hs], (1, B))).astype(NP_BF16),
        })

    res = run_bass_kernel_spmd(nc, in_maps, core_ids=list(range(NCORES)))

    y = np.empty((B, H, L), dtype=np.float32)
    for c in range(NCORES):
        yc = res.results[c]["y"].astype(np.float32)  # [HC, 128, 512]
        yc = yc.reshape(HC, B, 8, JH, T)             # (h, b, jl, jh, tau)
        yc = yc.transpose(1, 0, 3, 2, 4).reshape(B, HC, L)  # l=(jh,jl,tau)
        y[:, c * HC:(c + 1) * HC, :] = yc
    inv = np.empty(B, dtype=np.int64)
    inv[perm] = np.arange(B)
    y = y[inv]
    # np.where (not multiply): skipped store regions may hold garbage
    # bits (incl. NaN) when output buffers arrive non-zeroed
    y = np.where(mask[:, None, :], y, np.float32(0.0))
    return np.ascontiguousarray(y)


# revision 31
# speedup vs baseline: 1.0542x; 1.0095x over previous
"""DSSM (S4D-style) FFT-convolution kernel for Trainium2, 8 NeuronCores.

Math: y[b,h,:] = causal_conv(u_masked[b,h,:], K[h,:]) + D[h]*u_masked, masked,
where K[h,l] = 2*Re(sum_n Cs[h,n] * w[h,n]^l), w = exp(dt*A).

Algorithm (chunked state-space, T=256, J=16 chunks, N=64 complex states):
  intra-chunk:  lower-triangular Toeplitz matmul with K[0:256] (D folded in)
  inter-chunk:  A_j = V^T u_j (Vandermonde projection), S_j = w^T.S_{j-1} + A_j
                (complex scan over 16 steps on the vector engine, bf16),
                y_inter = Wout^T S_{j-1}
Sharding: H=256 channels split across 8 cores (32 each). Host does masking,
batch length-sorting, layout transforms, bf16 casts, and final unshard+mask.
Ragged lengths: batch sorted by length desc; scan steps and output stores
slice/skip the dead suffix.
"""

import numpy as np
import ml_dtypes

import concourse.bass as bass
import concourse.bacc as bacc
import concourse.mybir as mybir
import concourse.tile as tile
from concourse.bass_utils import run_bass_kernel_spmd

H, N, B, L = 256, 64, 16, 4096
NCORES = 8
HC = H // NCORES            # 32 channels per core
T, J = 256, 16              # chunk length, number of chunks
JH = 2                      # j-halves (8 chunks each) for 128-row tiles
N2 = 2 * N                  # 128 (real+imag state rows)
HG = 8                      # h-groups for pipelined input DMA

F32 = mybir.dt.float32
BF16 = mybir.dt.bfloat16
NP_BF16 = ml_dtypes.bfloat16


def _build_program(k_b):
    """k_b: per-(sorted)batch chunk counts, used to skip dead work at trace
    time. Correctness does not depend on them (host masks dead regions)."""
    # number of batches needing scan step j (producing S_j, used by chunk j+1)
    nb_scan = [sum(1 for k in k_b if k > j + 1) for j in range(J)]
    # number of batches with any alive chunk in half jh
    nb_half = [sum(1 for k in k_b if k > jh * 8) for jh in range(JH)]

    nc = bacc.Bacc("TRN2", target_bir_lowering=False, debug=False,
                   enable_asserts=False, num_devices=NCORES)

    HCG = HC // HG
    u_d = nc.dram_tensor("u_arr", [128, HC * 2 * J * B], BF16,
                         kind="ExternalInput")
    v_d = nc.dram_tensor("vwts", [128, HC * 256], BF16, kind="ExternalInput")
    w_d = nc.dram_tensor("wts", [128, HC * 512], BF16, kind="ExternalInput")
    pre_d = nc.dram_tensor("p_re", [128, HC * B], BF16, kind="ExternalInput")
    pim_d = nc.dram_tensor("p_im_s", [128, HC * B], BF16,
                           kind="ExternalInput")
    y_d = nc.dram_tensor("y", [HC, 128, JH * T], BF16, kind="ExternalOutput")

    with tile.TileContext(nc) as tc:
        with (
            tc.tile_pool(name="const", bufs=1) as cpool,
            tc.tile_pool(name="scantmp", bufs=3) as spool,
            tc.tile_pool(name="ysb", bufs=8) as ypool,
            tc.tile_pool(name="psum", bufs=8, space="PSUM") as psum,
        ):
            u_t = cpool.tile([128, HC * 2 * J * B], BF16, name="u_t")
            v_t = cpool.tile([128, HC * 256], BF16, name="v_t")
            w_t = cpool.tile([128, HC * 512], BF16, name="w_t")
            pre_t = cpool.tile([128, HC * B], BF16, name="pre_t")
            pim_t = cpool.tile([128, HC * B], BF16, name="pim_t")
            a_all = cpool.tile([128, HC * J * B], BF16, name="a_all")
            s_all = cpool.tile([128, (J + 1) * HC * B], BF16,
                               name="s_all")
            s_yout = cpool.tile([128, HC * J * B], BF16, name="s_yout")

            # split input DMAs by h-group so phase A can start early;
            # phase-C weights (Qrow/Wout) arrive last
            ucols = HCG * 2 * J * B
            vcols = HCG * 256
            for g in range(HG):
                nc.sync.dma_start(v_t[:, g * vcols:(g + 1) * vcols],
                                  v_d[:, g * vcols:(g + 1) * vcols])
                nc.sync.dma_start(u_t[:, g * ucols:(g + 1) * ucols],
                                  u_d[:, g * ucols:(g + 1) * ucols])
            nc.sync.dma_start(pre_t[:], pre_d[:])
            nc.sync.dma_start(pim_t[:], pim_d[:])
            wcols = HCG * 512
            for g in range(HG):
                nc.sync.dma_start(w_t[:, g * wcols:(g + 1) * wcols],
                                  w_d[:, g * wcols:(g + 1) * wcols])

            # views: a_all/s_all are SLOT-major (slot, h, b) so every scan
            # op is contiguous step-1 (2x/4x DVE modes); s_yout is
            # (h, jh, b, jl) for contiguous phase-C lhsT quarters
            SW = HC * B
            a_sl = a_all[:].rearrange("p (sl b h) -> p sl b h", sl=J, b=B)
            sy_v = s_yout[:].rearrange("p (h jh b jl) -> p h jh b jl",
                                       h=HC, jh=JH, b=B)

            def vslice(h, lo, hi):
                return v_t[:, h * 256 + lo:h * 256 + hi]

            def wslice(h, lo, hi):
                return w_t[:, h * 512 + lo:h * 512 + hi]

            # zero states (slot 0 = S_{-1}; also keeps skipped slots clean)
            nc.gpsimd.memset(s_all[:], 0.0)
            nc.gpsimd.memset(s_yout[:], 0.0)

            # ---- Phase A: state input projections A_j = V^T u_j ----
            for hp in range(HC // 2):
                a_ps = psum.tile([128, 2 * J * B], F32, name="a_ps",
                                 tag="ps")
                for i in range(2):
                    h = 2 * hp + i
                    nc.tensor.matmul(
                        a_ps[:, i * J * B:(i + 1) * J * B],
                        vslice(h, 0, 128),
                        u_t[:, (h * 2 + 0) * J * B:(h * 2 + 1) * J * B],
                        start=True, stop=False, skip_group_check=(i == 1))
                    nc.tensor.matmul(
                        a_ps[:, i * J * B:(i + 1) * J * B],
                        vslice(h, 128, 256),
                        u_t[:, (h * 2 + 1) * J * B:(h * 2 + 2) * J * B],
                        start=False, stop=True, skip_group_check=(i == 1))
                # scatter psum cols (i, jh, b, jl) into slot-major a_all;
                # split per (i, jh) to stay within 3D access patterns
                for i in range(2):
                    for jjh in range(JH):
                        a_out = a_sl[:, jjh * 8:(jjh + 1) * 8, :,
                                     2 * hp + i].rearrange("p jl b -> p b jl")
                        a_in = a_ps[:, i * 256 + jjh * 128:
                                    i * 256 + jjh * 128 + 128].rearrange(
                            "p (b jl) -> p b jl", b=B)
                        if (i + jjh) % 2 == 0:
                            nc.scalar.copy(a_out, a_in)
                        else:
                            nc.vector.tensor_copy(a_out, a_in)

            # ---- Phase B: complex scan S_j = p * S_{j-1} + A_j (bf16) ----
            for j in range(J):
                if nb_scan[j] == 0:
                    continue
                w_ = 32 * nb_scan[j]
                s_in = s_all[:, j * SW:j * SW + w_]
                s_out = s_all[:, (j + 1) * SW:(j + 1) * SW + w_]
                m_a = spool.tile([128, SW], BF16, name="m_a")
                swp = spool.tile([128, SW], BF16, name="swp")
                m_b = spool.tile([128, SW], BF16, name="m_b")
                tt = spool.tile([128, SW], BF16, name="tt")
                nc.vector.tensor_mul(m_a[:, 0:w_], pre_t[:, 0:w_], s_in)
                nc.gpsimd.tensor_copy(swp[0:64, 0:w_], s_in[64:128])
                nc.gpsimd.tensor_copy(swp[64:128, 0:w_], s_in[0:64])
                nc.vector.tensor_mul(m_b[:, 0:w_], pim_t[:, 0:w_], swp[:, 0:w_])
                nc.vector.tensor_add(tt[:, 0:w_], m_a[:, 0:w_], m_b[:, 0:w_])
                nc.vector.tensor_add(s_out, tt[:, 0:w_],
                                     a_all[:, j * SW:j * SW + w_])
                # off-chain: mirror the new slot into the yout layout
                jh_w, jl_w = divmod(j + 1, 8)
                if j + 1 < J:
                    nc.scalar.copy(
                        sy_v[:, :, jh_w, 0:nb_scan[j], jl_w],
                        s_out.rearrange("p (b h) -> p h b", b=nb_scan[j]))

            # ---- Phase C: intra Toeplitz + state output projection ----
            # stores merged 4 h at a time (dma_start issue cost dominates
            # many small stores)
            for jh in range(JH):
                rows = min(128, 8 * nb_half[jh])
                if rows == 0:
                    continue
                for hg in range(HC // 4):
                    y_sb = ypool.tile([128, 4, T], BF16, name="y_sb")
                    for i in range(4):
                        h = hg * 4 + i
                        y_ps = psum.tile([128, T], F32, name="y_ps",
                                         tag="ps")
                        # lhsT cols ordered (b, jl) -> out partitions b*8+jl
                        uo0 = ((h * 2 + 0) * 2 + jh) * 128
                        uo1 = ((h * 2 + 1) * 2 + jh) * 128
                        so = (h * 2 + jh) * 128
                        nc.tensor.matmul(y_ps[:], u_t[:, uo0:uo0 + 128],
                                         wslice(h, 0, 256),
                                         start=True, stop=False)
                        nc.tensor.matmul(y_ps[:, 128:256],
                                         u_t[:, uo1:uo1 + 128],
                                         wslice(h, 0, 128),
                                         start=False, stop=False,
                                         skip_group_check=True)
                        nc.tensor.matmul(y_ps[:], s_yout[:, so:so + 128],
                                         wslice(h, 256, 512),
                                         start=False, stop=True,
                                         skip_group_check=True)
                        if (h + jh) % 2 == 0:
                            nc.scalar.copy(y_sb[0:rows, i, :], y_ps[0:rows])
                        else:
                            nc.vector.tensor_copy(y_sb[0:rows, i, :],
                                                  y_ps[0:rows])
                    # dst iteration (row, h, tau) to match src layout
                    dst = y_d[hg * 4:hg * 4 + 4, 0:rows,
                              jh * T:(jh + 1) * T].rearrange(
                                  "h r t -> r h t")
                    nc.sync.dma_start(dst, y_sb[0:rows])

    nc.compile()
    return nc


_CACHE = {}


def _get_program(k_b):
    key = tuple(k_b)
    if key not in _CACHE:
        _CACHE[key] = _build_program(k_b)
    return _CACHE[key]


def _host_precompute(log_dt, C, log_A_real, A_imag, D):
    """Returns per-h weight blocks (fp64 internally)."""
    dt = np.exp(log_dt.astype(np.float64))
    A = -np.exp(log_A_real.astype(np.float64)) + 1j * A_imag.astype(np.float64)
    dtA = A * dt[:, None]
    w = np.exp(dtA)                                   # (H,N)
    Cc = C[..., 0].astype(np.float64) + 1j * C[..., 1].astype(np.float64)
    Cs = Cc * (np.exp(dtA) - 1.0) / A                 # (H,N)

    l = np.arange(T, dtype=np.float64)
    K = 2.0 * np.einsum('hn,hnl->hl', Cs, np.exp(dtA[:, :, None] * l)).real
    K[:, 0] += D.astype(np.float64)

    sig = np.arange(T)
    Vc = w[:, None, :] ** (T - sig)[None, :, None]    # (H,T,N)
    V_real = np.concatenate([Vc.real, Vc.imag], axis=2)  # (H,T,2N)

    tau = np.arange(T)
    Wc = Cs[:, :, None] * w[:, :, None] ** tau        # (H,N,T)
    W_real = np.concatenate([2 * Wc.real, -2 * Wc.imag], axis=1)  # (H,2N,T)

    p = w ** T                                        # (H,N)

    # Qrow0[h, sig', tau] = K[h, tau - sig'] for tau >= sig' else 0
    sp = np.arange(128)
    d = tau[None, :] - sp[:, None]                    # (128, 256)
    Qrow0 = np.where((d >= 0)[None], K[:, np.clip(d, 0, T - 1)], 0.0)
    return Qrow0, V_real, W_real, p


def kernel(u, length, log_dt, C, log_A_real, A_imag, D, **_unused):
    u = np.asarray(u, dtype=np.float32)
    length = np.asarray(length).astype(np.int64)
    mask = (np.arange(L)[None, :] < length[:, None])
    u_m = (u * mask[:, None, :]).astype(np.float32)

    # sort batches by length desc (stable) so dead work is a suffix
    perm = np.argsort(-length, kind="stable")
    k_b = [int(min(J, (int(length[b]) + T - 1) // T)) for b in perm]

    Qrow0, V_real, W_real, p = _host_precompute(
        np.asarray(log_dt), np.asarray(C), np.asarray(log_A_real),
        np.asarray(A_imag), np.asarray(D))

    # phase-A weights per h: [128, 256] = [V0 | V1]; phase-C: [Qrow0 | Wout]
    vwts = np.empty((H, 128, 256), dtype=np.float64)
    vwts[:, :, 0:128] = V_real[:, 0:128, :]      # lhsT [sig', n2]
    vwts[:, :, 128:256] = V_real[:, 128:256, :]
    vwts = vwts.astype(NP_BF16)
    wts = np.empty((H, 128, 512), dtype=np.float64)
    wts[:, :, 0:256] = Qrow0
    wts[:, :, 256:512] = W_real                  # rhs [n2, tau]
    wts = wts.astype(NP_BF16)

    # p tiles: [n2, (h,b)] with re duplicated on both halves; im sign-split
    p_re = np.empty((128, H), dtype=np.float32)
    p_im = np.empty((128, H), dtype=np.float32)
    p_re[0:64] = p.real.T
    p_re[64:128] = p.real.T
    p_im[0:64] = -p.imag.T
    p_im[64:128] = p.imag.T

    # u layout: (b,h,jh,jl,sb,sig') -> [sig', h, sb, jh, b_sorted, jl]
    u_s = u_m[perm]
    u_r = u_s.reshape(B, H, JH, 8, 2, 128).transpose(5, 1, 4, 2, 0, 3)
    u_bf = np.ascontiguousarray(u_r).astype(NP_BF16)

    nc = _get_program(k_b)
    in_maps = []
    for c in range(NCORES):
        hs = slice(c * HC, (c + 1) * HC)
        in_maps.append({
            "u_arr": np.ascontiguousarray(
                u_bf[:, hs].reshape(128, HC * 2 * J * B)),
            "vwts": np.ascontiguousarray(
                vwts[hs].transpose(1, 0, 2).reshape(128, HC * 256)),
            "wts": np.ascontiguousarray(
                wts[hs].transpose(1, 0, 2).reshape(128, HC * 512)),
            "p_re": np.ascontiguousarray(
                np.tile(p_re[:, hs], (1, B))).astype(NP_BF16),
            "p_im_s": np.ascontiguousarray(
                np.tile(p_im[:, hs], (1, B))).astype(NP_BF16),
        })

    res = run_bass_kernel_spmd(nc, in_maps, core_ids=list(range(NCORES)))

    y = np.empty((B, H, L), dtype=np.float32)
    for c in range(NCORES):
        yc = res.results[c]["y"].astype(np.float32)  # [HC, 128, 512]
        yc = yc.reshape(HC, B, 8, JH, T)             # (h, b, jl, jh, tau)
        yc = yc.transpose(1, 0, 3, 2, 4).reshape(B, HC, L)  # l=(jh,jl,tau)
        y[:, c * HC:(c + 1) * HC, :] = yc
    inv = np.empty(B, dtype=np.int64)
    inv[perm] = np.arange(B)
    y = y[inv]
    # np.where (not multiply): skipped store regions may hold garbage
    # bits (incl. NaN) when output buffers arrive non-zeroed
    y = np.where(mask[:, None, :], y, np.float32(0.0))
    return np.ascontiguousarray(y)
